# revision 26
# baseline (speedup 1.0000x reference)
"""Causal self-attention (fused QKV + RoPE + causal softmax + out-proj) on 8
Trainium2 NeuronCores.

Sharding: tensor-parallel by heads (2 heads/core, column-parallel c_attn,
causal attention per head, per-(batch,head) AllToAll reshard to row-split,
row-parallel c_proj).

v3 structure (all matmul operands bf16, fp32 PSUM):
  - Emission is software-pipelined ACROSS phases: batch-0 attention units are
    interleaved between phase-1 QKV strips (strips 4-7 compute batch-1 QKV
    while batch-0 attention runs), and proj(batch0) units are injected into
    batch-1 attention. The PE stream never drains, so the Tensor engine stays
    at its fast p-state, and the scalar engine's exp work (the second-largest
    engine load) overlaps QKV instead of serializing after it.
  - One shared 8-bank PSUM budget: qkv accumulators (3) + S-strips (2,
    double-buffered) + oacc half-pair (2) + 1 spare; the softmax-denominator
    broadcast borrows an S-strip buffer, proj borrows the qkv tags.
  - Attention processes each head in q-halves (oacc = 2 banks): per 128-key
    block, S^T pieces start at the diagonal and split on the global 512 grid,
    so AV accumulation never crosses a PSUM bank.
  - Softmax: exp on ACT (bf16 out), diagonal mask = bf16 multiply with a
    precomputed triangle (DVE 2x mode), denominators ride as ones-columns of
    the V slots, normalization = broadcast denom by tiny PE matmul, then
    reciprocal on 64 partitions (DVE), then multiply.
  - V^T is produced by DMA-xbar transposes (bf16), no PE transposes.
  - x / weights / A2A-scatter move as single large strided DMAs.
  - b_attn / b_proj are zero for this problem: default module skips bias
    work entirely; a bias-capable variant compiles lazily if needed.
"""

import sys

sys.path.insert(0, "/opt/trn_rl_repo")

from collections import deque

import numpy as np

import concourse.bass as bass
import concourse.mybir as mybir
import concourse.tile as tile
from concourse import bacc
from concourse.bass_utils import run_bass_kernel_spmd

B, T, C = 2, 2048, 1024
H, HD = 16, 64
HALF = HD // 2  # 32
NCORES = 8
HPC = H // NCORES  # 2 heads per core
ROWS = B * T  # 4096
DH = HPC * HD  # 128 channels per core
RPB = T // NCORES  # 256 rows per (core, batch)
ROPE_BASE = 10000.0
DT = mybir.dt.float32
BF = mybir.dt.bfloat16
FP = np.float32
NPBF = np.dtype(mybir.dt.np(BF))

KB = T // 128  # 16 key blocks per batch
NCI = C // 128  # 8 contraction chunks
P1C = 512  # phase-1 strip width (1 PSUM bank)
NSTRIP = ROWS // P1C  # 8
VW = 160  # V slot: [h0 0:64 | ones@64 | pad | h1 80:144 | ones@144 | pad]
# (xbar-transpose out offsets must be 16-element aligned -> h1 at 80)


def _build_module(use_collective=True, use_bias=False, debug=False):
    nc = bacc.Bacc("TRN2", target_bir_lowering=False, debug=False,
                   num_devices=NCORES)

    xT_t = nc.dram_tensor("xT", [C, ROWS], BF, kind="ExternalInput")
    # [1024, 384] = [wq | wk | wv] columns for this core's 2 heads
    w3_t = nc.dram_tensor("w3", [C, 3 * DH], BF, kind="ExternalInput")
    wp_t = nc.dram_tensor("wp", [C, C], BF, kind="ExternalInput")
    ones_t = nc.dram_tensor("ones512", [1, 512], BF, kind="ExternalInput")
    ropeC_t = nc.dram_tensor("ropeC", [DH, ROWS], BF, kind="ExternalInput")
    ropeS_t = nc.dram_tensor("ropeS", [DH, ROWS], BF, kind="ExternalInput")
    if use_bias:
        b3_t = nc.dram_tensor("b3", [1, 3 * DH], BF, kind="ExternalInput")
        bp_t = nc.dram_tensor("bp", [1, C], BF, kind="ExternalInput")
    # rows 0:RPB = batch-0 rows [RPB*c, RPB*(c+1)), rows RPB: = batch-1 same
    out_t = nc.dram_tensor("out", [2 * RPB, C], DT, kind="ExternalOutput")
    if debug:
        qT_d = nc.dram_tensor("qT_d", [DH, ROWS], BF, kind="ExternalOutput")
        kT_d = nc.dram_tensor("kT_d", [DH, ROWS], BF, kind="ExternalOutput")
        V_d = nc.dram_tensor("V_d", [128, 2 * KB * VW], BF,
                             kind="ExternalOutput")
        yT_d = nc.dram_tensor("yT_d", [DH, ROWS], BF, kind="ExternalOutput")
        yr_d = nc.dram_tensor("yr_d", [128, B * NCORES * RPB], BF,
                              kind="ExternalOutput")

    SCALE = 1.0 / float(np.sqrt(HD))

    with tile.TileContext(nc) as tc, nc.allow_low_precision(
            reason="bf16 kernel: matmul operands and intermediates are bf16"):
        with (
            tc.tile_pool(name="persist", bufs=1) as pp,
            tc.tile_pool(name="dram", bufs=1, space="DRAM") as dp,
            tc.tile_pool(name="psum", bufs=1, space="PSUM") as ps,
            tc.tile_pool(name="xs", bufs=3) as xsp,
            tc.tile_pool(name="work", bufs=1) as wk,
        ):
            # x strip prefetches: first loads on the DMA critical path
            xs_tiles = {}

            def prefetch(Q):
                xs = xsp.tile([128, NCI, P1C], BF, tag="xs")
                nc.sync.dma_start(
                    xs[:],
                    xT_t[:, Q * P1C:(Q + 1) * P1C].rearrange(
                        "(ci p) c -> p ci c", p=128))
                xs_tiles[Q] = xs

            w3 = pp.tile([128, NCI, 3 * DH], BF, tag="w3")
            C_sb = pp.tile([DH, ROWS], BF, tag="ropeC")
            S_sb = pp.tile([DH, ROWS], BF, tag="ropeS")
            dum_in = dp.tile([NCORES, 1, 2], BF, tag="dum_in", name="dum_in")
            dum_out = dp.tile([NCORES, 1, 2], BF, tag="dum_out",
                              name="dum_out")
            if use_collective:
                # tiny warmup AllToAll first: pays the collective setup cost
                # and aligns the cores before phase 1 queues anything big
                nc.sync.dma_start(dum_in[:], ones_t[:, 0:16])
                nc.gpsimd.collective_compute(
                    "AllToAll", mybir.AluOpType.bypass,
                    replica_groups=[list(range(NCORES))],
                    ins=[dum_in.opt()], outs=[dum_out.opt()])
            nc.sync.dma_start(
                w3[:], w3_t[:].rearrange("(ci p) m -> p ci m", p=128))
            prefetch(0)
            nc.scalar.dma_start(C_sb[:, 0:T], ropeC_t[:, 0:T])
            nc.scalar.dma_start(S_sb[:, 0:T], ropeS_t[:, 0:T])
            prefetch(1)
            ones_row = pp.tile([1, 512], BF, tag="ones_row")
            nc.sync.dma_start(ones_row[:], ones_t[:])

            tri = pp.tile([128, 128], BF, tag="tri")
            nc.vector.memset(tri[:], 1.0)
            nc.gpsimd.affine_select(
                out=tri[:], in_=tri[:], compare_op=mybir.AluOpType.is_ge,
                fill=0.0, base=0, pattern=[[1, 128]], channel_multiplier=-1)

            wp_sb = pp.tile([128, NCI, C], BF, tag="wp")
            if use_bias:
                b3 = pp.tile([1, 3 * DH], BF, tag="b3")
                bp = pp.tile([1, C], BF, tag="bp")
                nc.sync.dma_start(b3[:], b3_t[:])
                nc.sync.dma_start(bp[:], bp_t[:])

            a2a_in = [[dp.tile([NCORES, HD, RPB], BF, tag=f"a2a_in{b}{h}",
                               name=f"a2a_in{b}{h}") for h in range(HPC)]
                      for b in range(B)]
            a2a_out = [[dp.tile([NCORES, HD, RPB], BF, tag=f"a2a_out{b}{h}",
                                name=f"a2a_out{b}{h}") for h in range(HPC)]
                       for b in range(B)]
            yr = [pp.tile([128, NCORES, RPB], BF, tag=f"yr{b}",
                          name=f"yr{b}") for b in range(B)]

            qT = pp.tile([DH, ROWS], BF, tag="qT")
            kT = pp.tile([DH, ROWS], BF, tag="kT")
            V_all = pp.tile([128, 2 * KB, VW], BF, tag="V_all")
            # ones columns at 64 (head0 lhsT col 64) and 144 (head1 lhsT
            # col 64): both heads get denom at out row 64, channels at 0:64
            nc.vector.memset(V_all[:, :, 64:65], 1.0)
            nc.vector.memset(V_all[:, :, 144:145], 1.0)
            yT = pp.tile([DH, ROWS], BF, tag="yT")


            delayed = deque()

            def drain_one():
                if delayed:
                    delayed.popleft()()

            def drain_all():
                while delayed:
                    delayed.popleft()()

            # ---------------- phase 1: one QKV+rope strip -----------------
            PART = [1, 0, 3, 2]  # rope half-rotation partner groups

            def strip_qk(Q):
                with nc.named_scope("qkv"):
                    cols = slice(Q * P1C, (Q + 1) * P1C)
                    xs = xs_tiles[Q]
                    qps = ps.tile([128, P1C], DT, tag="qps")
                    kps = ps.tile([128, P1C], DT, tag="kps")
                    for ci in range(NCI):
                        st = ci == 0
                        sp = (ci == NCI - 1) and not use_bias
                        nc.tensor.matmul(qps[:], w3[:, ci, 0:128],
                                         xs[:, ci, :], start=st, stop=sp)
                        nc.tensor.matmul(kps[:], w3[:, ci, 128:256],
                                         xs[:, ci, :], start=st, stop=sp)
                    if use_bias:
                        nc.tensor.matmul(qps[:], b3[:, 0:128], ones_row[:],
                                         start=False, stop=True)
                        nc.tensor.matmul(kps[:], b3[:, 128:256], ones_row[:],
                                         start=False, stop=True)
                    if Q + 2 < NSTRIP:
                        prefetch(Q + 2)

                    # rope reads the accumulators straight from PSUM
                    # (the partition-offset rot-mults need one PSUM operand:
                    # SBUF+SBUF tensor_tensor requires equal base partitions)
                    for src_t, dst in ((qps, qT), (kps, kT)):
                        ta = wk.tile([128, P1C], DT, tag="ta", bufs=2)
                        tb_ = wk.tile([128, P1C], DT, tag="tb", bufs=2)
                        nc.vector.tensor_tensor(
                            ta[:], src_t[:], C_sb[:, cols],
                            mybir.AluOpType.mult)
                        for g in range(4):
                            gs = slice(32 * g, 32 * g + 32)
                            prt = slice(32 * PART[g], 32 * PART[g] + 32)
                            nc.vector.tensor_tensor(
                                tb_[gs, :], src_t[prt, :], S_sb[gs, cols],
                                mybir.AluOpType.mult)
                        nc.gpsimd.tensor_tensor(
                            dst[:, cols], ta[:], tb_[:], mybir.AluOpType.add)

            def strip_v(Q):
                with nc.named_scope("qkv"):
                    xs = xs_tiles.pop(Q)
                    # V pass reuses the q accumulator bank (rope has read it
                    # by then); keeps phase-1 PSUM at 2 banks
                    vps = ps.tile([128, P1C], DT, tag="qps", name="vps")
                    for ci in range(NCI):
                        nc.tensor.matmul(vps[:], w3[:, ci, 256:384],
                                         xs[:, ci, :], start=(ci == 0),
                                         stop=(ci == NCI - 1) and not use_bias)
                    if use_bias:
                        nc.tensor.matmul(vps[:], b3[:, 256:384], ones_row[:],
                                         start=False, stop=True)

                    # V: PSUM -> bf16 (per-head half-height tiles so the
                    # xbar transpose inputs sit at partition base 0), then
                    # DMA-xbar transposes into each head's V slot
                    vt0 = wk.tile([HD, P1C], BF, tag="vt0", bufs=2)
                    vt1 = wk.tile([HD, P1C], BF, tag="vt1", bufs=2)
                    nc.scalar.copy(vt0[:], vps[0:HD, :])
                    nc.scalar.copy(vt1[:], vps[HD:128, :])
                    for tb in range(P1C // 128):
                        gtb = Q * (P1C // 128) + tb
                        sl = slice(tb * 128, (tb + 1) * 128)
                        nc.scalar.dma_start_transpose(
                            out=V_all[:, gtb, 0:HD], in_=vt0[:, sl])
                        nc.sync.dma_start_transpose(
                            out=V_all[:, gtb, 80:80 + HD], in_=vt1[:, sl])

            # ---------------- phase 2: attention units --------------------
            def make_chunk(sps_u, lo_u, hi_u, kb_u, b_u, h_u, oacc_u):
                def run():
                    w = hi_u - lo_u
                    psb = wk.tile([128, 1024], BF, tag="psb", bufs=4,
                                  name="psb")
                    nc.scalar.activation(
                        psb[:, 0:w], sps_u[:, 0:w],
                        mybir.ActivationFunctionType.Exp, scale=SCALE)
                    if lo_u == kb_u * 128:
                        nc.vector.tensor_tensor(
                            psb[:, 0:128], psb[:, 0:128], tri[:],
                            mybir.AluOpType.mult)
                    vloc = V_all[:, b_u * KB + kb_u,
                                 80 * h_u:80 * h_u + 65]
                    g0 = lo_u
                    while g0 < hi_u:
                        qb = g0 // 512
                        g1 = min(hi_u, (qb + 1) * 512)
                        nc.tensor.matmul(
                            oacc_u[qb % 2][:, g0 - qb * 512:g1 - qb * 512],
                            vloc, psb[:, g0 - lo_u:g1 - lo_u],
                            start=(kb_u == 0), stop=(kb_u == 4 * qb + 3))
                        g0 = g1
                return run

            def make_finalize(oacc_u, b_u, h_u, qb_u):
                hp_u = slice(h_u * HD, (h_u + 1) * HD)
                dr = HD  # denom row (both heads)
                c0 = 0

                def fin():
                    dsb = wk.tile([1, 512], BF, tag="dsb", bufs=3,
                                  name="dsb")
                    nc.vector.tensor_copy(dsb[:], oacc_u[dr:dr + 1, :])
                    # broadcast the denom row to 64 partitions via a tiny
                    # matmul; borrow an S-strip PSUM buffer
                    rps = ps.tile([128, 1024], DT, tag="sps", bufs=2,
                                  name="rps")
                    nc.tensor.matmul(rps[0:HD, 0:512], ones_row[:, 0:HD],
                                     dsb[:], start=True, stop=True)
                    rsb = wk.tile([128, 512], DT, tag="rsb", bufs=3,
                                  name="rsb")
                    nc.vector.reciprocal_approx_fast(rsb[0:HD, :],
                                                     rps[0:HD, 0:512])
                    nc.vector.tensor_tensor(
                        yT[hp_u, b_u * T + qb_u * 512:
                           b_u * T + (qb_u + 1) * 512],
                        oacc_u[c0:c0 + HD, :], rsb[0:HD, :],
                        mybir.AluOpType.mult)
                return fin

            def make_a2a(b_u, h_u):
                hp_u = slice(h_u * HD, (h_u + 1) * HD)

                def coll():
                    for j in range(NCORES):
                        nc.sync.dma_start(
                            a2a_in[b_u][h_u][j],
                            yT[hp_u, b_u * T + j * RPB:
                               b_u * T + (j + 1) * RPB])
                    if use_collective:
                        nc.gpsimd.collective_compute(
                            "AllToAll", mybir.AluOpType.bypass,
                            replica_groups=[list(range(NCORES))],
                            ins=[a2a_in[b_u][h_u].opt()],
                            outs=[a2a_out[b_u][h_u].opt()])
                    else:
                        nc.sync.dma_start(a2a_out[b_u][h_u][:],
                                          a2a_in[b_u][h_u][:])

                def scatter():
                    nc.sync.dma_start(
                        yr[b_u][HD * h_u:HD * (h_u + 1), :, :],
                        a2a_out[b_u][h_u][:].rearrange("j p c -> p j c"))
                return coll, scatter

            def attn_units(b):
                """List of emission thunks: per (head, q-half, key-block)."""
                units = []
                bT = b * T
                for half in range(2):
                    for h in range(HPC):
                        hp = slice(h * HD, (h + 1) * HD)
                        qlo, qhi = half * 1024, half * 1024 + 1024
                        oacc = {}

                        def kb_visit(kb, h=h, hp=hp, half=half, qlo=qlo,
                                     qhi=qhi, oacc=oacc, last=False):
                            def run():
                                with nc.named_scope("attn"):
                                    if not oacc:
                                        for i in range(2):
                                            oacc[i] = ps.tile(
                                                [HD + 1, 512], DT,
                                                tag=f"oacc{i}",
                                                name=f"oacc{b}{h}{half}{i}")
                                    qs = kb * 128
                                    lhs_k = kT[hp, bT + qs:bT + qs + 128]
                                    lo = max(qs, qlo)
                                    sps = ps.tile([128, 1024], DT,
                                                  tag="sps", bufs=2,
                                                  name="sps")
                                    # S pieces split on the sps-tile 512
                                    # grid (PSUM bank boundary)
                                    a = lo
                                    while a < qhi:
                                        e = min(qhi,
                                                lo + ((a - lo) // 512 + 1)
                                                * 512)
                                        nc.tensor.matmul(
                                            sps[:, a - lo:e - lo], lhs_k,
                                            qT[hp, bT + a:bT + e],
                                            start=True, stop=True)
                                        a = e
                                    while len(delayed) > 2:
                                        drain_one()
                                    delayed.append(make_chunk(
                                        sps, lo, qhi, kb, b, h, oacc))
                                    if last:
                                        for qb in (half * 2, half * 2 + 1):
                                            delayed.append(make_finalize(
                                                oacc[qb % 2], b, h, qb))
                            return run

                        kbs = range(8) if half == 0 else range(16)
                        for kb in kbs:
                            units.append(kb_visit(kb, last=(kb == kbs[-1])))
                stages = [make_a2a(b, h) for h in range(HPC)]
                return units, stages

            # ---------------- phase 3: proj units -------------------------
            def proj_part(b, tb, co, pps, r0, r1, start, stop):
                for s in range(NCI):
                    nc.tensor.matmul(
                        pps[:],
                        yr[b][r0:r1, s, tb * 128:(tb + 1) * 128],
                        wp_sb[r0:r1, s, co * 512:(co + 1) * 512],
                        start=(start and s == 0),
                        stop=(stop and s == NCI - 1) and not use_bias)

            def proj_out(b, tb, co, pps):
                if use_bias:
                    nc.tensor.matmul(pps[:], ones_row[:, 0:128],
                                     bp[:, co * 512:(co + 1) * 512],
                                     start=False, stop=True)
                osb = wk.tile([128, 512], DT, tag="osb", bufs=4, name="osb")
                nc.vector.tensor_copy(osb[:], pps[:])
                nc.sync.dma_start(
                    out_t[b * RPB + tb * 128:b * RPB + (tb + 1) * 128,
                          co * 512:(co + 1) * 512], osb[:])

            def proj_units(b):
                units = []
                for i, (tb, co) in enumerate(
                        (tb, co) for tb in range(2) for co in range(2)):
                    def unit(tb=tb, co=co, i=i):
                        def run():
                            with nc.named_scope("proj"):
                                pps = ps.tile([128, 512], DT,
                                              tag=("qps", "kps")[i % 2],
                                              name=f"pps{b}{tb}{co}")
                                proj_part(b, tb, co, pps, 0, 128, True, True)
                                proj_out(b, tb, co, pps)
                        return run
                    units.append(unit())
                return units

            def proj_hsplit(b):
                # all h0-slot partial sums first (they only need the
                # first-head A2A), then h1 parts once the last collective
                # lands; 4 live accumulators via the freed oacc tags
                with nc.named_scope("proj"):
                    tags = ("qps", "kps", "oacc0", "oacc1")
                    ppss = []
                    for i, (tb, co) in enumerate(
                            (tb, co) for tb in range(2) for co in range(2)):
                        pps = ps.tile([128, 512], DT, tag=tags[i],
                                      name=f"pps{b}{tb}{co}")
                        proj_part(b, tb, co, pps, 0, HD, True, False)
                        ppss.append((tb, co, pps))
                    for tb, co, pps in ppss:
                        proj_part(b, tb, co, pps, HD, 128, False, True)
                        proj_out(b, tb, co, pps)

            # ---------------- master schedule -----------------------------
            # warm the PE p-state with ~3us of tiny matmuls so the first
            # strips run at full clock
            wps_ = ps.tile([128, 1024], DT, tag="sps", bufs=2, name="wps_")
            for i in range(40):
                nc.tensor.matmul(wps_[0:HD, 0:64], ones_row[:, 0:HD],
                                 ones_row[:, 0:64], start=(i == 0),
                                 stop=(i == 39))
            strip_qk(0)
            strip_v(0)
            strip_qk(1)
            strip_v(1)
            u0, st0 = attn_units(0)
            k = 0
            per_strip = {2: 0, 3: 4, 4: 4, 5: 4, 6: 4, 7: 4}
            for Q in range(2, NSTRIP):
                strip_qk(Q)
                if Q == 2:
                    nc.scalar.dma_start(C_sb[:, T:ROWS], ropeC_t[:, T:ROWS])
                    nc.scalar.dma_start(S_sb[:, T:ROWS], ropeS_t[:, T:ROWS])
                if Q == 3:
                    # w_proj load once the startup DMA burst is over
                    nc.sync.dma_start(
                        wp_sb[:],
                        wp_t[:].rearrange("(ci p) m -> p ci m", p=128))
                for _ in range(per_strip[Q]):
                    u0[k]()
                    k += 1
                strip_v(Q)
                for _ in range(per_strip[Q]):
                    u0[k]()
                    k += 1
            while k < len(u0):
                u0[k]()
                k += 1
            drain_all()
            st0[0][0]()  # collective (b0, h0)
            st0[1][0]()  # collective (b0, h1)

            u1, st1 = attn_units(1)
            p0 = proj_units(0)
            inject = {33: 0, 39: 1, 43: 2}
            for i, u in enumerate(u1):
                u()
                if i == 25:
                    st0[0][1]()  # scatter (b0, h0) - collective long done
                    st0[1][1]()  # scatter (b0, h1)
                if i == 31:
                    drain_all()
                    st1[0][0]()  # collective (b1, h0)
                if i in inject:
                    p0[inject[i]]()
            drain_all()
            st1[1][0]()  # collective (b1, h1)
            st1[0][1]()  # scatter (b1, h0)
            p0[3]()
            st1[1][1]()  # scatter (b1, h1) - the exposed tail
            proj_hsplit(1)

            if debug:
                nc.sync.dma_start(qT_d[:], qT[:])
                nc.sync.dma_start(kT_d[:], kT[:])
                nc.sync.dma_start(V_d[:], V_all[:].rearrange("p a b -> p (a b)"))
                nc.sync.dma_start(yT_d[:], yT[:])
                nc.sync.dma_start(
                    yr_d[:, 0:NCORES * RPB],
                    yr[0][:].rearrange("p a b -> p (a b)"))
                nc.sync.dma_start(
                    yr_d[:, NCORES * RPB:],
                    yr[1][:].rearrange("p a b -> p (a b)"))

    nc.compile()
    return nc


_NC_CACHE = {}


def _get_module(use_bias):
    key = bool(use_bias)
    if key not in _NC_CACHE:
        _NC_CACHE[key] = _build_module(use_bias=key)
    return _NC_CACHE[key]


def _rope_tables():
    inv = ROPE_BASE ** (-np.arange(HALF, dtype=np.float64) / HALF)
    tt = np.arange(T, dtype=np.float64)
    ang = tt[None, :] * inv[:, None]  # [32, T]
    cos = np.cos(ang).astype(FP)
    sin = np.sin(ang).astype(FP)
    Cq = np.concatenate([cos, cos], axis=0)  # [64, T] (p%32 freq)
    Sq = np.concatenate([-sin, sin], axis=0)
    Cq = np.tile(Cq, (HPC, B))
    Sq = np.tile(Sq, (HPC, B))
    return (np.ascontiguousarray(Cq).astype(NPBF),
            np.ascontiguousarray(Sq).astype(NPBF))


def kernel(x, w_attn, b_attn, w_proj, b_proj, _trace=False):
    x = np.asarray(x, dtype=FP)
    w_attn = np.asarray(w_attn, dtype=FP)
    b_attn = np.asarray(b_attn, dtype=FP)
    w_proj = np.asarray(w_proj, dtype=FP)
    b_proj = np.asarray(b_proj, dtype=FP)
    use_bias = bool(np.any(b_attn) or np.any(b_proj))

    xT = np.ascontiguousarray(x.reshape(ROWS, C).T).astype(NPBF)
    ropeC, ropeS = _rope_tables()
    ones512 = np.ones((1, 512), NPBF)
    wp = w_proj.astype(NPBF)

    in_maps = []
    for c in range(NCORES):
        h0 = HPC * c
        cols = slice(h0 * HD, (h0 + HPC) * HD)  # this core's head channels
        w3 = np.concatenate(
            [w_attn[:, i * C:(i + 1) * C][:, cols] for i in range(3)],
            axis=1).astype(NPBF)
        m = {
            "xT": xT,
            "w3": np.ascontiguousarray(w3),
            "wp": wp,
            "ones512": ones512,
            "ropeC": ropeC,
            "ropeS": ropeS,
        }
        if use_bias:
            b3 = np.concatenate(
                [b_attn[i * C:(i + 1) * C][cols] for i in range(3)])
            m["b3"] = np.ascontiguousarray(b3[None, :]).astype(NPBF)
            m["bp"] = np.ascontiguousarray(b_proj[None, :]).astype(NPBF)
        in_maps.append(m)

    nc = _get_module(use_bias)
    res = run_bass_kernel_spmd(nc, in_maps, core_ids=list(range(NCORES)),
                               trace=_trace)
    out = np.empty((B, T, C), dtype=FP)
    for c in range(NCORES):
        o = res.results[c]["out"]
        for b in range(B):
            out[b, RPB * c:RPB * (c + 1), :] = o[b * RPB:(b + 1) * RPB]
    if _trace:
        kernel.last_results = res
    return out
